# revision 15
# baseline (speedup 1.0000x reference)
"""Trainium2 Bass kernel for nn_Bilinear_54065048322517.

Math:  out[b, j] = input2[b, j] * sum_{i,k} weights[i, j, k] * input1[b, i]
           =   input2 * (input1 @ weights.sum(axis=2))
Shapes: input1 (16384, 64) f32, input2 (16384, 2048) f32,
        weights (64, 2048, 64) f32, out (16384, 2048) f32.

Sharding: split J=2048 into 8 shards of 256 (one per NeuronCore);
J-sharding avoids replicating the 32MB weights tensor.  bf16 on the
HBM side: 20MB traffic per core, ~48us at the ~420GB/s sustained
per-core HBM rate.

v3 pipeline (rebuilt from per-core ntff analysis of v1/v2):
  - The k-reduce of w happens mostly IN THE DMA: w is staged as 4
    passes of [128, 2048] bf16 whose element-wise sum is the
    16-way-partial k-sum; passes 1-3 use SWDGE accum_op=add onto the
    first.  One 2048-elem DVE tensor_reduce finishes the job.  This
    replaces v1's 9.6us serial DVE reduce chain (bf16 4-term
    accumulation adds ~0.2% relative error, well inside the budget).
  - All loads are triggered upfront with dep-free buffers (x2 pool
    holds all 8 groups): loads can never wait behind store-sem waits
    (v1's straggler cores lost ~10us to exactly that).
  - x1 (head cols first) + all x2 groups stream on the scalar HWDGE
    ring; the sync ring runs the sel-mask load then stores only.
    A single ring saturates the ~420GB/s HBM pipe, so ring balance
    does not matter; FIFO order (loads before stores) does.
  - Main loop keeps v1's proven 1024-col psum pairs (ps bufs=3,
    ACT evac stream runs dense), but every 4th pair skips the ACT
    evac: DVE multiplies straight from PSUM f32 (1x rate).  That
    balances ACT (~25us) and DVE (~28us) streams.
  - Stores are half-group (512KB, 16 of them) so the tail after the
    last multiply is ~1.2us and the back-half bus stays fed.
"""

import numpy as np

B, I, J, K = 16384, 64, 2048, 64
NCORES = 8
JS = J // NCORES          # 256 columns per core
NG = 8                    # groups; each group = 2048 B-rows = [128, 4096]
GFREE = 4096              # x2/out free elems per partition per group
NPAIR = 32                # psum pair-tiles of 1024 cols
X1HEAD = 2048             # first x1 cols loaded separately (groups 0-1)
NWCHUNK = 8               # w chunk loads (reduce granularity)
DIRECT_EVERY = 4          # every 4th pair is DVE-direct from PSUM

_CACHE = {}


def _build_nc():
    from contextlib import ExitStack

    import concourse.mybir as mybir
    import concourse.tile as tile
    from concourse import bacc

    f32 = mybir.dt.float32
    bf16 = mybir.dt.bfloat16
    nc = bacc.Bacc()

    x1 = nc.dram_tensor("input1", [128, 64 * 128], bf16, kind="ExternalInput")
    x2 = nc.dram_tensor("input2", [NG, 128, GFREE], bf16, kind="ExternalInput")
    w = nc.dram_tensor("weights", [128, 128 * K], bf16, kind="ExternalInput")
    seld = nc.dram_tensor("sel", [128, 256], bf16, kind="ExternalInput")
    out = nc.dram_tensor("out", [NG, 128, GFREE], bf16, kind="ExternalOutput")

    with tile.TileContext(nc) as tc, ExitStack() as ctx:
        const_pool = ctx.enter_context(tc.tile_pool(name="const", bufs=1))
        wc_pool = ctx.enter_context(tc.tile_pool(name="wc", bufs=1))
        x_pool = ctx.enter_context(tc.tile_pool(name="xin", bufs=NG))
        o_pool = ctx.enter_context(tc.tile_pool(name="oout", bufs=NG))
        yb_pool = ctx.enter_context(tc.tile_pool(name="yb", bufs=8))
        ps_pool = ctx.enter_context(tc.tile_pool(name="ps", bufs=3, space="PSUM"))
        tr_pool = ctx.enter_context(tc.tile_pool(name="tr", bufs=1, space="PSUM"))

        # ---- sel + all w chunks head the SYNC ring (the scalar ring's
        # head is blocked ~1.5us by the implicit ACT table load) ----
        selt = const_pool.tile([128, 256], bf16, name="sel")
        nc.sync.dma_start(out=selt[:], in_=seld[:])
        w2h = const_pool.tile([128, 128], bf16)  # partition h*64+i, col j''
        wcsz = 128 * K // NWCHUNK  # 1024 elems/partition/chunk
        jcs = 128 // NWCHUNK       # 16 w2h cols per chunk
        wchunks = []
        for c in range(NWCHUNK):
            wchunk = wc_pool.tile([128, wcsz], bf16, name=f"wchunk{c}", tag=f"wc{c}")
            wchunks.append(wchunk)
            nc.sync.dma_start(out=wchunk[:], in_=w[:, c * wcsz : (c + 1) * wcsz])

        # ---- x1 + early x2 groups on scalar; LATE x2 groups on sync
        # AHEAD of the stores, so stores can never starve them ----
        x1T = const_pool.tile([128, 64 * 128], bf16)
        nc.scalar.dma_start(out=x1T[:, 0:X1HEAD], in_=x1[:, 0:X1HEAD])
        nc.scalar.dma_start(out=x1T[:, X1HEAD:], in_=x1[:, X1HEAD:])
        xtiles = []
        for g in range(NG):
            xt = x_pool.tile([128, GFREE], bf16, name=f"xt{g}", tag="xt")
            xtiles.append(xt)
            eng = nc.scalar if g < 4 else nc.sync
            eng.dma_start(out=xt[:], in_=x2[g])

        # ---- k-reduce: serial DVE chain, pipelined under chunk loads ----
        with nc.allow_low_precision("w2 reduce rounds only on the bf16 store"):
            for c in range(NWCHUNK):
                nc.vector.tensor_reduce(
                    out=w2h[:, c * jcs : (c + 1) * jcs],
                    in_=wchunks[c][:].rearrange("p (j k) -> p j k", k=K),
                    axis=mybir.AxisListType.X,
                    op=mybir.AluOpType.add,
                )

        # ---- de-interleave + duplicate w2 via selection matmuls ----
        # pdup[q*64+i, h*128+j''] = w2h[h*64+i, j'']
        pdup = tr_pool.tile([128, 512], f32, name="pdup")
        for h in range(2):
            nc.tensor.matmul(
                pdup[:, h * 128 : (h + 1) * 128],
                lhsT=selt[:, h * 128 : (h + 1) * 128],
                rhs=w2h[:],
                start=True,
                stop=True,
            )
        w2dup = const_pool.tile([128, JS], bf16)
        nc.scalar.copy(w2dup[:], pdup[:, 0:JS])

        # ---- main loop: 32 pair-tiles of 1024 psum cols ----
        # pair m covers supertiles n = 2m, 2m+1; psum col layout
        # q*512 + u*256 + j; x2/out col c = s*512 + q*256 + j with
        # s = m%4*2 + u  (group g = m//4).
        def pair_matmuls(m):
            pt = ps_pool.tile([128, 1024], f32, tag="ps")
            for u in range(2):
                n = 2 * m + u
                for q in range(2):
                    nc.tensor.matmul(
                        pt[:, q * 512 + u * 256 : q * 512 + (u + 1) * 256],
                        lhsT=x1T[q * 64 : (q + 1) * 64, n * 128 : (n + 1) * 128],
                        rhs=w2dup[q * 64 : (q + 1) * 64, :],
                        start=True,
                        stop=True,
                    )
            return pt

        for m in range(NPAIR):
            g, mm = m // 4, m % 4
            pt = pair_matmuls(m)
            if mm == 0:
                ot = o_pool.tile([128, GFREE], bf16, name=f"ot{g}", tag="ot")
            sl = slice(mm * 1024, (mm + 1) * 1024)
            ot_v = ot[:, sl].rearrange("p (u q j) -> p q u j", u=2, q=2)
            xt_v = xtiles[g][:, sl].rearrange("p (u q j) -> p q u j", u=2, q=2)
            pt_v = pt[:].rearrange("p (q u j) -> p q u j", q=2, u=2)
            if m % DIRECT_EVERY == DIRECT_EVERY - 1:
                nc.vector.tensor_mul(ot_v, pt_v, xt_v)
            else:
                yb = yb_pool.tile([128, 1024], bf16, name="yb", tag="yb")
                nc.scalar.copy(yb[:], pt[:])
                nc.vector.tensor_mul(
                    ot_v, yb[:].rearrange("p (q u j) -> p q u j", q=2, u=2), xt_v
                )
            if mm % 2 == 1:
                half = mm // 2
                nc.sync.dma_start(
                    out=out[g][:, half * 2048 : (half + 1) * 2048],
                    in_=ot[:, half * 2048 : (half + 1) * 2048],
                )

    nc.compile()
    return nc


def _get_nc():
    if "nc" not in _CACHE:
        _CACHE["nc"] = _build_nc()
    return _CACHE["nc"]


def _make_in_maps(input1, input2, weights):
    import ml_dtypes

    BF = ml_dtypes.bfloat16
    input1 = np.asarray(input1, dtype=np.float32)
    input2 = np.asarray(input2, dtype=np.float32)
    weights = np.asarray(weights, dtype=np.float32)

    # x1t[q*64+i, n*128+p] = input1[n*256 + 2p + q, i]
    x1t = (
        input1.reshape(64, 128, 2, I)
        .transpose(2, 3, 0, 1)
        .reshape(128, 64 * 128)
        .astype(BF)
    )

    # sel[h*64+i, h*128 + q*64+i] = 1
    sel = np.zeros((128, 256), dtype=BF)
    idx = np.arange(64)
    for h in range(2):
        for q in range(2):
            sel[h * 64 + idx, h * 128 + q * 64 + idx] = 1.0

    in_maps = []
    for c in range(NCORES):
        sl = slice(c * JS, (c + 1) * JS)
        # wd[h*64+i, j''*64+k] = weights[i, c*JS + h*128 + j'', k]
        wd = (
            weights[:, sl, :]
            .reshape(I, 2, 128, K)
            .transpose(1, 0, 2, 3)
            .reshape(128, 128 * K)
            .astype(BF)
        )
        # x2d[g, p, (s*2+q)*256+j] = input2[(g*8+s)*256 + 2p + q, sl][j]
        x2d = (
            input2[:, sl]
            .reshape(NG, 8, 128, 2, JS)
            .transpose(0, 2, 1, 3, 4)
            .reshape(NG, 128, GFREE)
            .astype(BF)
        )
        in_maps.append({"input1": x1t, "input2": x2d, "weights": wd, "sel": sel})
    return in_maps


def run(input1, input2, weights, trace=False, **spmd_kwargs):
    from concourse.bass_utils import run_bass_kernel_spmd

    nc = _get_nc()
    in_maps = _make_in_maps(input1, input2, weights)
    res = run_bass_kernel_spmd(
        nc, in_maps, core_ids=list(range(NCORES)), trace=trace, **spmd_kwargs
    )
    outs = []
    for c in range(NCORES):
        o = np.asarray(res.results[c]["out"])  # (NG, 128, GFREE) bf16
        outs.append(
            o.reshape(NG, 128, 8, 2, JS)
            .transpose(0, 2, 1, 3, 4)
            .reshape(B, JS)
        )
    full = np.concatenate(outs, axis=1).astype(np.float32)
    return full, res


def kernel(input1, input2, weights):
    full, _ = run(input1, input2, weights, trace=False)
    return full


# revision 18
# speedup vs baseline: 1.0112x; 1.0112x over previous
"""Trainium2 Bass kernel for nn_Bilinear_54065048322517.

Math:  out[b, j] = input2[b, j] * sum_{i,k} weights[i, j, k] * input1[b, i]
           =   input2 * (input1 @ weights.sum(axis=2))
Shapes: input1 (16384, 64) f32, input2 (16384, 2048) f32,
        weights (64, 2048, 64) f32, out (16384, 2048) f32.

Sharding: split J=2048 into 8 shards of 256 (one per NeuronCore);
J-sharding avoids replicating the 32MB weights tensor.  bf16 on the
HBM side: 20MB traffic per core, ~48us at the ~420GB/s sustained
per-core HBM rate.

v3 pipeline (rebuilt from per-core ntff analysis of v1/v2):
  - The k-reduce of w happens mostly IN THE DMA: w is staged as 4
    passes of [128, 2048] bf16 whose element-wise sum is the
    16-way-partial k-sum; passes 1-3 use SWDGE accum_op=add onto the
    first.  One 2048-elem DVE tensor_reduce finishes the job.  This
    replaces v1's 9.6us serial DVE reduce chain (bf16 4-term
    accumulation adds ~0.2% relative error, well inside the budget).
  - All loads are triggered upfront with dep-free buffers (x2 pool
    holds all 8 groups): loads can never wait behind store-sem waits
    (v1's straggler cores lost ~10us to exactly that).
  - x1 (head cols first) + all x2 groups stream on the scalar HWDGE
    ring; the sync ring runs the sel-mask load then stores only.
    A single ring saturates the ~420GB/s HBM pipe, so ring balance
    does not matter; FIFO order (loads before stores) does.
  - Main loop keeps v1's proven 1024-col psum pairs (ps bufs=3,
    ACT evac stream runs dense), but every 4th pair skips the ACT
    evac: DVE multiplies straight from PSUM f32 (1x rate).  That
    balances ACT (~25us) and DVE (~28us) streams.
  - Stores are half-group (512KB, 16 of them) so the tail after the
    last multiply is ~1.2us and the back-half bus stays fed.
"""

import numpy as np

B, I, J, K = 16384, 64, 2048, 64
NCORES = 8
JS = J // NCORES          # 256 columns per core
NG = 8                    # groups; each group = 2048 B-rows = [128, 4096]
GFREE = 4096              # x2/out free elems per partition per group
NPAIR = 32                # psum pair-tiles of 1024 cols
X1HEAD = 2048             # first x1 cols loaded separately (groups 0-1)
NWCHUNK = 8               # w chunk loads (reduce granularity)
DIRECT_EVERY = 4          # every 4th pair is DVE-direct from PSUM

_CACHE = {}


def _build_nc():
    from contextlib import ExitStack

    import concourse.mybir as mybir
    import concourse.tile as tile
    from concourse import bacc

    f32 = mybir.dt.float32
    bf16 = mybir.dt.bfloat16
    nc = bacc.Bacc()

    class LeanTileContext(tile.TileContext):
        """TileContext whose exit skips the ~6us end-of-kernel semaphore
        RANGE_CLEAR walk + second barrier.  Safe here because every
        kernel() call loads the NEFF fresh (NRT zeroes semaphores at
        load) and executes it once; the clear only matters when
        re-executing an already-loaded NEFF."""

        def _drain_and_barrier(self, tick_clock, wait_clock):
            from concourse.tile import ScopedClock

            drain_inst = self.nc.sync.drain()
            wait_clock.add_sem_waits(
                drain_inst.ins, ScopedClock({None: tick_clock.global_clock})
            )
            self.nc.all_engine_barrier()
            popped = self.nc._tile_sem_poison_stack.pop()
            assert popped is self._sem_poison

    x1 = nc.dram_tensor("input1", [128, 64 * 128], bf16, kind="ExternalInput")
    x2 = nc.dram_tensor("input2", [NG, 128, GFREE], bf16, kind="ExternalInput")
    w = nc.dram_tensor("weights", [128, 128 * K], bf16, kind="ExternalInput")
    seld = nc.dram_tensor("sel", [128, 256], bf16, kind="ExternalInput")
    out = nc.dram_tensor("out", [NG, 128, GFREE], bf16, kind="ExternalOutput")

    with LeanTileContext(nc) as tc, ExitStack() as ctx:
        const_pool = ctx.enter_context(tc.tile_pool(name="const", bufs=1))
        wc_pool = ctx.enter_context(tc.tile_pool(name="wc", bufs=1))
        x_pool = ctx.enter_context(tc.tile_pool(name="xin", bufs=NG))
        o_pool = ctx.enter_context(tc.tile_pool(name="oout", bufs=NG))
        yb_pool = ctx.enter_context(tc.tile_pool(name="yb", bufs=8))
        ps_pool = ctx.enter_context(tc.tile_pool(name="ps", bufs=3, space="PSUM"))
        tr_pool = ctx.enter_context(tc.tile_pool(name="tr", bufs=1, space="PSUM"))

        # ---- w chunks head BOTH rings (evens scalar, odds sync) so the
        # chunk-landing rate stays ahead of the serial DVE reduce chain ----
        w2h = const_pool.tile([128, 128], bf16)  # partition h*64+i, col j''
        wcsz = 128 * K // NWCHUNK  # 1024 elems/partition/chunk
        jcs = 128 // NWCHUNK       # 16 w2h cols per chunk
        wchunks = []
        for c in range(NWCHUNK):
            wchunk = wc_pool.tile([128, wcsz], bf16, name=f"wchunk{c}", tag=f"wc{c}")
            wchunks.append(wchunk)
            eng = nc.scalar if c % 2 == 0 else nc.sync
            eng.dma_start(out=wchunk[:], in_=w[:, c * wcsz : (c + 1) * wcsz])
        selt = const_pool.tile([128, 256], bf16, name="sel")
        nc.sync.dma_start(out=selt[:], in_=seld[:])

        # ---- x1 + early x2 groups on scalar; LATE x2 groups on sync
        # AHEAD of the stores, so stores can never starve them ----
        x1T = const_pool.tile([128, 64 * 128], bf16)
        nc.scalar.dma_start(out=x1T[:, 0:X1HEAD], in_=x1[:, 0:X1HEAD])
        nc.scalar.dma_start(out=x1T[:, X1HEAD:], in_=x1[:, X1HEAD:])
        xtiles = []
        for g in range(NG):
            xt = x_pool.tile([128, GFREE], bf16, name=f"xt{g}", tag="xt")
            xtiles.append(xt)
            eng = nc.scalar if g < 4 else nc.sync
            eng.dma_start(out=xt[:], in_=x2[g])

        # ---- k-reduce: serial DVE chain, pipelined under chunk loads ----
        with nc.allow_low_precision("w2 reduce rounds only on the bf16 store"):
            for c in range(NWCHUNK):
                nc.vector.tensor_reduce(
                    out=w2h[:, c * jcs : (c + 1) * jcs],
                    in_=wchunks[c][:].rearrange("p (j k) -> p j k", k=K),
                    axis=mybir.AxisListType.X,
                    op=mybir.AluOpType.add,
                )

        # ---- de-interleave + duplicate w2 via selection matmuls ----
        # pdup[q*64+i, h*128+j''] = w2h[h*64+i, j'']
        pdup = tr_pool.tile([128, 512], f32, name="pdup")
        for h in range(2):
            nc.tensor.matmul(
                pdup[:, h * 128 : (h + 1) * 128],
                lhsT=selt[:, h * 128 : (h + 1) * 128],
                rhs=w2h[:],
                start=True,
                stop=True,
            )
        w2dup = const_pool.tile([128, JS], bf16)
        nc.scalar.copy(w2dup[:], pdup[:, 0:JS])

        # ---- main loop: 32 pair-tiles of 1024 psum cols ----
        # pair m covers supertiles n = 2m, 2m+1; psum col layout
        # q*512 + u*256 + j; x2/out col c = s*512 + q*256 + j with
        # s = m%4*2 + u  (group g = m//4).
        def pair_matmuls(m):
            pt = ps_pool.tile([128, 1024], f32, tag="ps")
            for u in range(2):
                n = 2 * m + u
                for q in range(2):
                    nc.tensor.matmul(
                        pt[:, q * 512 + u * 256 : q * 512 + (u + 1) * 256],
                        lhsT=x1T[q * 64 : (q + 1) * 64, n * 128 : (n + 1) * 128],
                        rhs=w2dup[q * 64 : (q + 1) * 64, :],
                        start=True,
                        stop=True,
                    )
            return pt

        for m in range(NPAIR):
            g, mm = m // 4, m % 4
            pt = pair_matmuls(m)
            if mm == 0:
                ot = o_pool.tile([128, GFREE], bf16, name=f"ot{g}", tag="ot")
            sl = slice(mm * 1024, (mm + 1) * 1024)
            ot_v = ot[:, sl].rearrange("p (u q j) -> p q u j", u=2, q=2)
            xt_v = xtiles[g][:, sl].rearrange("p (u q j) -> p q u j", u=2, q=2)
            pt_v = pt[:].rearrange("p (q u j) -> p q u j", q=2, u=2)
            if m % DIRECT_EVERY == DIRECT_EVERY - 1:
                nc.vector.tensor_mul(ot_v, pt_v, xt_v)
            else:
                yb = yb_pool.tile([128, 1024], bf16, name="yb", tag="yb")
                nc.scalar.copy(yb[:], pt[:])
                nc.vector.tensor_mul(
                    ot_v, yb[:].rearrange("p (q u j) -> p q u j", q=2, u=2), xt_v
                )
            if mm % 2 == 1:
                half = mm // 2
                nc.sync.dma_start(
                    out=out[g][:, half * 2048 : (half + 1) * 2048],
                    in_=ot[:, half * 2048 : (half + 1) * 2048],
                )

    nc.compile()
    return nc


def _get_nc():
    if "nc" not in _CACHE:
        _CACHE["nc"] = _build_nc()
    return _CACHE["nc"]


def _make_in_maps(input1, input2, weights):
    import ml_dtypes

    BF = ml_dtypes.bfloat16
    input1 = np.asarray(input1, dtype=np.float32)
    input2 = np.asarray(input2, dtype=np.float32)
    weights = np.asarray(weights, dtype=np.float32)

    # x1t[q*64+i, n*128+p] = input1[n*256 + 2p + q, i]
    x1t = (
        input1.reshape(64, 128, 2, I)
        .transpose(2, 3, 0, 1)
        .reshape(128, 64 * 128)
        .astype(BF)
    )

    # sel[h*64+i, h*128 + q*64+i] = 1
    sel = np.zeros((128, 256), dtype=BF)
    idx = np.arange(64)
    for h in range(2):
        for q in range(2):
            sel[h * 64 + idx, h * 128 + q * 64 + idx] = 1.0

    in_maps = []
    for c in range(NCORES):
        sl = slice(c * JS, (c + 1) * JS)
        # wd[h*64+i, j''*64+k] = weights[i, c*JS + h*128 + j'', k]
        wd = (
            weights[:, sl, :]
            .reshape(I, 2, 128, K)
            .transpose(1, 0, 2, 3)
            .reshape(128, 128 * K)
            .astype(BF)
        )
        # x2d[g, p, (s*2+q)*256+j] = input2[(g*8+s)*256 + 2p + q, sl][j]
        x2d = (
            input2[:, sl]
            .reshape(NG, 8, 128, 2, JS)
            .transpose(0, 2, 1, 3, 4)
            .reshape(NG, 128, GFREE)
            .astype(BF)
        )
        in_maps.append({"input1": x1t, "input2": x2d, "weights": wd, "sel": sel})
    return in_maps


def run(input1, input2, weights, trace=False, **spmd_kwargs):
    from concourse.bass_utils import run_bass_kernel_spmd

    nc = _get_nc()
    in_maps = _make_in_maps(input1, input2, weights)
    res = run_bass_kernel_spmd(
        nc, in_maps, core_ids=list(range(NCORES)), trace=trace, **spmd_kwargs
    )
    outs = []
    for c in range(NCORES):
        o = np.asarray(res.results[c]["out"])  # (NG, 128, GFREE) bf16
        outs.append(
            o.reshape(NG, 128, 8, 2, JS)
            .transpose(0, 2, 1, 3, 4)
            .reshape(B, JS)
        )
    full = np.concatenate(outs, axis=1).astype(np.float32)
    return full, res


def kernel(input1, input2, weights):
    full, _ = run(input1, input2, weights, trace=False)
    return full


# revision 21
# speedup vs baseline: 1.0171x; 1.0059x over previous
"""Trainium2 Bass kernel for nn_Bilinear_54065048322517.

Math:  out[b, j] = input2[b, j] * sum_{i,k} weights[i, j, k] * input1[b, i]
           =   input2 * (input1 @ weights.sum(axis=2))
Shapes: input1 (16384, 64) f32, input2 (16384, 2048) f32,
        weights (64, 2048, 64) f32, out (16384, 2048) f32.

Sharding: split J=2048 into 8 shards of 256 (one per NeuronCore);
J-sharding avoids replicating the 32MB weights tensor.  bf16 on the
HBM side: 20MB traffic per core, ~48us at the ~420GB/s sustained
per-core HBM rate.

v3 pipeline (rebuilt from per-core ntff analysis of v1/v2):
  - The k-reduce of w happens mostly IN THE DMA: w is staged as 4
    passes of [128, 2048] bf16 whose element-wise sum is the
    16-way-partial k-sum; passes 1-3 use SWDGE accum_op=add onto the
    first.  One 2048-elem DVE tensor_reduce finishes the job.  This
    replaces v1's 9.6us serial DVE reduce chain (bf16 4-term
    accumulation adds ~0.2% relative error, well inside the budget).
  - All loads are triggered upfront with dep-free buffers (x2 pool
    holds all 8 groups): loads can never wait behind store-sem waits
    (v1's straggler cores lost ~10us to exactly that).
  - x1 (head cols first) + all x2 groups stream on the scalar HWDGE
    ring; the sync ring runs the sel-mask load then stores only.
    A single ring saturates the ~420GB/s HBM pipe, so ring balance
    does not matter; FIFO order (loads before stores) does.
  - Main loop keeps v1's proven 1024-col psum pairs (ps bufs=3,
    ACT evac stream runs dense), but every 4th pair skips the ACT
    evac: DVE multiplies straight from PSUM f32 (1x rate).  That
    balances ACT (~25us) and DVE (~28us) streams.
  - Stores are half-group (512KB, 16 of them) so the tail after the
    last multiply is ~1.2us and the back-half bus stays fed.
"""

import numpy as np

B, I, J, K = 16384, 64, 2048, 64
NCORES = 8
JS = J // NCORES          # 256 columns per core
NG = 8                    # groups; each group = 2048 B-rows = [128, 4096]
GFREE = 4096              # x2/out free elems per partition per group
NPAIR = 32                # psum pair-tiles of 1024 cols
X1HEAD = 2048             # first x1 cols loaded separately (groups 0-1)
NWCHUNK = 8               # w chunk loads (reduce granularity)
DIRECT_EVERY = 4          # every 4th pair is DVE-direct from PSUM
NWARM = 44                # PE HAM warm-up dummy matmuls

_CACHE = {}


def _build_nc():
    from contextlib import ExitStack

    import concourse.mybir as mybir
    import concourse.tile as tile
    from concourse import bacc

    f32 = mybir.dt.float32
    bf16 = mybir.dt.bfloat16
    nc = bacc.Bacc()

    class LeanTileContext(tile.TileContext):
        """TileContext whose exit skips the ~6us end-of-kernel semaphore
        RANGE_CLEAR walk + second barrier.  Safe here because every
        kernel() call loads the NEFF fresh (NRT zeroes semaphores at
        load) and executes it once; the clear only matters when
        re-executing an already-loaded NEFF."""

        def _drain_and_barrier(self, tick_clock, wait_clock):
            from concourse.tile import ScopedClock

            drain_inst = self.nc.sync.drain()
            wait_clock.add_sem_waits(
                drain_inst.ins, ScopedClock({None: tick_clock.global_clock})
            )
            self.nc.all_engine_barrier()
            popped = self.nc._tile_sem_poison_stack.pop()
            assert popped is self._sem_poison

    x1 = nc.dram_tensor("input1", [128, 64 * 128], bf16, kind="ExternalInput")
    x2 = nc.dram_tensor("input2", [NG, 128, GFREE], bf16, kind="ExternalInput")
    w = nc.dram_tensor("weights", [128, 128 * K], bf16, kind="ExternalInput")
    seld = nc.dram_tensor("sel", [128, 256], bf16, kind="ExternalInput")
    out = nc.dram_tensor("out", [NG, 128, GFREE], bf16, kind="ExternalOutput")

    with LeanTileContext(nc) as tc, ExitStack() as ctx:
        const_pool = ctx.enter_context(tc.tile_pool(name="const", bufs=1))
        wc_pool = ctx.enter_context(tc.tile_pool(name="wc", bufs=1))
        x_pool = ctx.enter_context(tc.tile_pool(name="xin", bufs=NG))
        o_pool = ctx.enter_context(tc.tile_pool(name="oout", bufs=NG))
        yb_pool = ctx.enter_context(tc.tile_pool(name="yb", bufs=8))
        ps_pool = ctx.enter_context(tc.tile_pool(name="ps", bufs=3, space="PSUM"))
        tr_pool = ctx.enter_context(tc.tile_pool(name="tr", bufs=1, space="PSUM"))

        # ---- sel first on sync (feeds the PE warm-up), then w chunks
        # head BOTH rings exclusively (evens scalar, odds sync) so the
        # chunk-landing rate stays ahead of the serial DVE reduce chain ----
        selt = const_pool.tile([128, 256], bf16, name="sel")
        nc.sync.dma_start(out=selt[:], in_=seld[:])
        w2h = const_pool.tile([128, 128], bf16)  # partition h*64+i, col j''
        wcsz = 128 * K // NWCHUNK  # 1024 elems/partition/chunk
        jcs = 128 // NWCHUNK       # 16 w2h cols per chunk
        wchunks = []
        for c in range(NWCHUNK):
            wchunk = wc_pool.tile([128, wcsz], bf16, name=f"wchunk{c}", tag=f"wc{c}")
            wchunks.append(wchunk)
            eng = nc.scalar if c % 2 == 0 else nc.sync
            eng.dma_start(out=wchunk[:], in_=w[:, c * wcsz : (c + 1) * wcsz])

        # ---- x1 behind the w evens + early x2 groups on scalar; LATE
        # x2 groups on sync AHEAD of the stores (stores can't starve) ----
        x1T = const_pool.tile([128, 64 * 128], bf16)
        nc.scalar.dma_start(out=x1T[:, 0:X1HEAD], in_=x1[:, 0:X1HEAD])
        nc.scalar.dma_start(out=x1T[:, X1HEAD:], in_=x1[:, X1HEAD:])
        xtiles = []
        for g in range(NG):
            xt = x_pool.tile([128, GFREE], bf16, name=f"xt{g}", tag="xt")
            xtiles.append(xt)
            eng = nc.scalar if g < 4 else nc.sync
            eng.dma_start(out=xt[:], in_=x2[g])

        # ---- PE HAM warm-up: dense dummy matmuls while the PE would
        # otherwise idle through the w-load+reduce head.  The PE's
        # activity monitor only unlocks the 2.4GHz clock after ~3.4us of
        # sustained busy; without this the whole matmul stream runs at
        # 1.2GHz (measured 245ns vs ~110ns per 256-col matmul). ----
        warm = ctx.enter_context(tc.tile_pool(name="warm", bufs=1, space="PSUM"))
        wpt = warm.tile([128, 512], f32, name="warmps")
        for _ in range(NWARM):
            nc.tensor.matmul(
                wpt[:, 0:256],
                lhsT=selt[:, 0:128],
                rhs=selt[:],
                start=True,
                stop=True,
            )

        # ---- k-reduce: serial DVE chain, pipelined under chunk loads ----
        with nc.allow_low_precision("w2 reduce rounds only on the bf16 store"):
            for c in range(NWCHUNK):
                nc.vector.tensor_reduce(
                    out=w2h[:, c * jcs : (c + 1) * jcs],
                    in_=wchunks[c][:].rearrange("p (j k) -> p j k", k=K),
                    axis=mybir.AxisListType.X,
                    op=mybir.AluOpType.add,
                )

        # ---- de-interleave + duplicate w2 via selection matmuls ----
        # pdup[q*64+i, h*128+j''] = w2h[h*64+i, j'']
        pdup = tr_pool.tile([128, 512], f32, name="pdup")
        for h in range(2):
            nc.tensor.matmul(
                pdup[:, h * 128 : (h + 1) * 128],
                lhsT=selt[:, h * 128 : (h + 1) * 128],
                rhs=w2h[:],
                start=True,
                stop=True,
            )
        w2dup = const_pool.tile([128, JS], bf16)
        nc.scalar.copy(w2dup[:], pdup[:, 0:JS])

        # ---- main loop: 32 pair-tiles of 1024 psum cols ----
        # pair m covers supertiles n = 2m, 2m+1; psum col layout
        # q*512 + u*256 + j; x2/out col c = s*512 + q*256 + j with
        # s = m%4*2 + u  (group g = m//4).
        def pair_matmuls(m):
            pt = ps_pool.tile([128, 1024], f32, tag="ps")
            for u in range(2):
                n = 2 * m + u
                for q in range(2):
                    nc.tensor.matmul(
                        pt[:, q * 512 + u * 256 : q * 512 + (u + 1) * 256],
                        lhsT=x1T[q * 64 : (q + 1) * 64, n * 128 : (n + 1) * 128],
                        rhs=w2dup[q * 64 : (q + 1) * 64, :],
                        start=True,
                        stop=True,
                    )
            return pt

        for m in range(NPAIR):
            g, mm = m // 4, m % 4
            pt = pair_matmuls(m)
            if mm == 0:
                ot = o_pool.tile([128, GFREE], bf16, name=f"ot{g}", tag="ot")
            sl = slice(mm * 1024, (mm + 1) * 1024)
            ot_v = ot[:, sl].rearrange("p (u q j) -> p q u j", u=2, q=2)
            xt_v = xtiles[g][:, sl].rearrange("p (u q j) -> p q u j", u=2, q=2)
            pt_v = pt[:].rearrange("p (q u j) -> p q u j", q=2, u=2)
            if m % DIRECT_EVERY == DIRECT_EVERY - 1:
                nc.vector.tensor_mul(ot_v, pt_v, xt_v)
            else:
                yb = yb_pool.tile([128, 1024], bf16, name="yb", tag="yb")
                nc.scalar.copy(yb[:], pt[:])
                nc.vector.tensor_mul(
                    ot_v, yb[:].rearrange("p (q u j) -> p q u j", q=2, u=2), xt_v
                )
            if g == NG - 1:
                # last group: store quarter-size so the final drain after
                # the last multiply is as short as possible
                nc.sync.dma_start(out=out[g][:, sl], in_=ot[:, sl])
            elif mm % 2 == 1:
                half = mm // 2
                nc.sync.dma_start(
                    out=out[g][:, half * 2048 : (half + 1) * 2048],
                    in_=ot[:, half * 2048 : (half + 1) * 2048],
                )

    nc.compile()
    return nc


def _get_nc():
    if "nc" not in _CACHE:
        _CACHE["nc"] = _build_nc()
    return _CACHE["nc"]


def _make_in_maps(input1, input2, weights):
    import ml_dtypes

    BF = ml_dtypes.bfloat16
    input1 = np.asarray(input1, dtype=np.float32)
    input2 = np.asarray(input2, dtype=np.float32)
    weights = np.asarray(weights, dtype=np.float32)

    # x1t[q*64+i, n*128+p] = input1[n*256 + 2p + q, i]
    x1t = (
        input1.reshape(64, 128, 2, I)
        .transpose(2, 3, 0, 1)
        .reshape(128, 64 * 128)
        .astype(BF)
    )

    # sel[h*64+i, h*128 + q*64+i] = 1
    sel = np.zeros((128, 256), dtype=BF)
    idx = np.arange(64)
    for h in range(2):
        for q in range(2):
            sel[h * 64 + idx, h * 128 + q * 64 + idx] = 1.0

    in_maps = []
    for c in range(NCORES):
        sl = slice(c * JS, (c + 1) * JS)
        # wd[h*64+i, j''*64+k] = weights[i, c*JS + h*128 + j'', k]
        wd = (
            weights[:, sl, :]
            .reshape(I, 2, 128, K)
            .transpose(1, 0, 2, 3)
            .reshape(128, 128 * K)
            .astype(BF)
        )
        # x2d[g, p, (s*2+q)*256+j] = input2[(g*8+s)*256 + 2p + q, sl][j]
        x2d = (
            input2[:, sl]
            .reshape(NG, 8, 128, 2, JS)
            .transpose(0, 2, 1, 3, 4)
            .reshape(NG, 128, GFREE)
            .astype(BF)
        )
        in_maps.append({"input1": x1t, "input2": x2d, "weights": wd, "sel": sel})
    return in_maps


def run(input1, input2, weights, trace=False, **spmd_kwargs):
    from concourse.bass_utils import run_bass_kernel_spmd

    nc = _get_nc()
    in_maps = _make_in_maps(input1, input2, weights)
    res = run_bass_kernel_spmd(
        nc, in_maps, core_ids=list(range(NCORES)), trace=trace, **spmd_kwargs
    )
    outs = []
    for c in range(NCORES):
        o = np.asarray(res.results[c]["out"])  # (NG, 128, GFREE) bf16
        outs.append(
            o.reshape(NG, 128, 8, 2, JS)
            .transpose(0, 2, 1, 3, 4)
            .reshape(B, JS)
        )
    full = np.concatenate(outs, axis=1).astype(np.float32)
    return full, res


def kernel(input1, input2, weights):
    full, _ = run(input1, input2, weights, trace=False)
    return full
